# revision 16
# baseline (speedup 1.0000x reference)
"""Causal self-attention (GPT-2 style) on 8 Trainium2 NeuronCores.

Sharding: tensor-parallel over heads. Each of the 8 cores owns 2 of the 16
heads: it computes q/k/v projections for its heads (column-sharded w_attn),
runs causal attention for them, and multiplies by its row-slice of w_proj,
producing a partial (E, B*T) output in bf16. The host sums the 8 partials.

Kernel structure (v6):
- All matmul operands are bf16 (full PE rate at any moving size, half the
  DMA bytes). PSUM accumulation stays fp32.
- V is produced directly in natural [token, dh] layout (lhsT = x chunk,
  rhs = w_v): no transposes/copies for V. The projection bias is folded in
  as a rank-1 ones @ bias matmul in the same accumulation group.
- Scores are key-major S^T = K Q^T ([key, query] tiles) with causal
  narrowing: a diagonal key block only computes the queries that can see it
  (widths 512/384/256/128).
- Off-diagonal score blocks are computed in PAIRS into [128,1024] PSUM
  tiles (two banks, one accumulation group per bank) so a single exp
  instruction covers both blocks (halves the per-instruction ACT overhead).
- AV is computed transposed: out[q, d] = et[:, q-subtile]^T @ V with
  queries on PSUM partitions. The softmax denominator (ones-column of V)
  is then per-PARTITION: normalization is reciprocal([128,1]) +
  tensor_scalar_mul. The four per-quarter accumulation groups share one
  PSUM bank; start is set on the bank's first matmul and stop on its last
  (HW zero-region semantics: start wipes the whole 2KB bank).
- Causal masking is a 0/1 upper-triangular multiply on the otherwise-idle
  GPSIMD engine applied to the 128-wide diagonal triangle of et (exp
  without max-subtraction is safe here; masked lanes become exactly 0).
- attn-out is transposed back ([128,128] bf16 PE transposes, deferred one
  round so PE never waits on the DVE norm-evict chain) into the [dh,
  token] layout phase C needs.
- Phase C partials are staged per 512-token chunk in SBUF (bf16) and
  stored with two DMAs per chunk.
- Emission follows an explicit round schedule that pairs ACT-heavy
  attention chunks (large causal prefixes) with PE-heavy projection
  chunks, processing batch 1's chunks in the order c1, c3, c2, c0 so the
  tail rounds are light.
"""

import numpy as np
from contextlib import ExitStack

import ml_dtypes
import concourse.bass as bass
import concourse.bacc as bacc
import concourse.mybir as mybir
import concourse.tile as tile
from concourse import bass_utils

F32 = mybir.dt.float32
BF16 = mybir.dt.bfloat16
AF = mybir.ActivationFunctionType

B, T, E = 2, 2048, 1024
NH, DH = 16, 64
NCORES = 8
HPC = NH // NCORES          # heads per core = 2
BT = B * T                  # 4096 tokens total
TCH = 512                   # token chunk
NTC = BT // TCH             # 8 token chunks
NE = E // 128               # 8 contraction tiles over E
ST = T // 128               # 16 key tiles per batch
CPB = T // TCH              # 4 query chunks per batch
SCALE = 1.0 / 8.0           # 1/sqrt(DH)


def _emit(tc, yT, xT, wqk, wv, bqk, bv, wp, bp, trid, identd):
    nc = tc.nc
    ctx = tc._ctx

    singles = ctx.enter_context(tc.tile_pool(name="singles", bufs=1))
    xpool = ctx.enter_context(tc.tile_pool(name="xpool", bufs=3))
    epool = ctx.enter_context(tc.tile_pool(name="epool", bufs=8))
    apool = ctx.enter_context(tc.tile_pool(name="apool", bufs=12))
    rpool = ctx.enter_context(tc.tile_pool(name="rpool", bufs=8))
    ypool = ctx.enter_context(tc.tile_pool(name="ypool", bufs=2))
    psum = ctx.enter_context(tc.tile_pool(name="psum", space="PSUM", bufs=2))

    # --- persistent tiles; only wqk is loaded up front (x0 races it) ---
    wqk_sb = singles.tile([128, NE, 2 * 128], BF16)
    nc.scalar.dma_start(out=wqk_sb,
                        in_=wqk.rearrange("(e p) m -> p e m", p=128))
    wv_sb = singles.tile([128, NE, 128], BF16)
    bqk_sb = singles.tile([128, 2], F32)
    bv_sb = singles.tile([1, 128], BF16)
    bp_sb = singles.tile([128, NE], F32)
    tri_sb = singles.tile([128, 128], BF16)
    id_sb = singles.tile([128, 128], BF16)
    wp_sb = singles.tile([128, E], BF16)
    ones_sb = singles.tile([1, 128], BF16)
    nc.gpsimd.memset(ones_sb, 1.0)

    def load_consts():
        nc.scalar.dma_start(out=wv_sb,
                            in_=wv.rearrange("(e p) m -> p e m", p=128))
        nc.scalar.dma_start(out=bqk_sb,
                            in_=bqk.rearrange("(c p) -> p c", p=128))
        nc.scalar.dma_start(out=bv_sb, in_=bv)
        nc.scalar.dma_start(out=tri_sb, in_=trid)
        nc.scalar.dma_start(out=id_sb, in_=identd)

    def load_wp():
        nc.scalar.dma_start(out=wp_sb, in_=wp)
        nc.scalar.dma_start(out=bp_sb, in_=bp.rearrange("(c p) -> p c", p=128))

    qT = singles.tile([128, BT], BF16)   # rows: 2 heads x 64 dh
    kT = singles.tile([128, BT], BF16)
    aoT = singles.tile([128, BT], BF16)  # normalized attn-out^T
    # V natural layout per (batch, s-tile, head): [s-tok, dh] + ones col 64
    v1 = singles.tile([128, B, ST, HPC, 65], BF16)
    nc.gpsimd.memset(v1[:, :, :, :, 64:65], 1.0)

    xchunks = [None] * NTC

    # ---------------- phase A units (projection of one 512-token chunk) ----
    def make_a_units(g):
        units = []
        halves = 2 if g == 0 else 1

        def load_x(lo, hi):
            def unit():
                if xchunks[g] is None:
                    xchunks[g] = xpool.tile([128, NE, TCH], BF16, tag="xch",
                                            name=f"xch{g}")
                nc.sync.dma_start(
                    out=xchunks[g][:, :, lo:hi],
                    in_=xT.rearrange("(e p) t -> p e t", p=128)[
                        :, :, g * TCH + lo:g * TCH + hi],
                )
            return unit

        def qk(m, lo, hi):
            def unit():
                xch = xchunks[g]
                ps = psum.tile([128, hi - lo], F32, tag="mm", bufs=4,
                               name=f"psA{g}_{m}_{lo}")
                for e in range(NE):
                    nc.tensor.matmul(
                        ps,
                        lhsT=wqk_sb[:, e, m * 128:(m + 1) * 128],
                        rhs=xch[:, e, lo:hi],
                        start=(e == 0),
                        stop=(e == NE - 1),
                    )
                dst = qT if m == 0 else kT
                sl = dst[:, g * TCH + lo:g * TCH + hi]
                if m == 0:
                    nc.scalar.activation(sl, ps, AF.Identity,
                                         bias=bqk_sb[:, 0:1])
                else:
                    nc.vector.tensor_scalar_add(sl, ps, bqk_sb[:, 1:2])
            return unit

        bidx, cc = divmod(g, CPB)

        def vtile(stl):
            def unit():
                xch = xchunks[g]
                s_idx = cc * 4 + stl
                ps = psum.tile([128, 2, 64], F32, tag="aux", bufs=2,
                               name=f"psV{g}_{stl}")
                for e in range(NE):
                    nc.tensor.matmul(
                        ps,
                        lhsT=xch[:, e, stl * 128:(stl + 1) * 128],
                        rhs=wv_sb[:, e, :],
                        start=(e == 0), stop=False,
                    )
                # += ones^T (1x128) @ bv (1x128): broadcast bias over tokens
                nc.tensor.matmul(ps, lhsT=ones_sb, rhs=bv_sb,
                                 start=False, stop=True)
                if stl % 2 == 0:
                    nc.scalar.copy(v1[:, bidx, s_idx, :, 0:64], ps)
                else:
                    nc.vector.tensor_copy(v1[:, bidx, s_idx, :, 0:64], ps)
            return unit

        if halves == 2:
            units.append(load_x(0, 256))
            units.append(load_consts)
            units.append(qk(0, 0, 256))
            units.append(load_x(256, 512))
            units.append(qk(1, 0, 256))
            units.append(vtile(0))
            units.append(vtile(1))
            units.append(qk(0, 256, 512))
            units.append(qk(1, 256, 512))
            units.append(vtile(2))
            units.append(vtile(3))
        else:
            units.append(load_x(0, 512))
            units.append(qk(0, 0, 512))
            units.append(qk(1, 0, 512))
            for stl in range(4):
                units.append(vtile(stl))
        return units

    # ---------------- phase B units (attention for one chunk) --------------
    def make_b_units(g):
        bidx, cc = divmod(g, CPB)
        units = []
        smax = 4 * cc + 3
        ao_nat = [None] * 4  # per q-subtile [128, HPC, 64] bf16

        def alloc_ao(j):
            def unit():
                ao_nat[j] = apool.tile([128, HPC, 64], BF16, tag="ao",
                                       name=f"ao{g}_{j}")
            return unit

        def head_work(hh):
            hs = slice(hh * 64, (hh + 1) * 64)
            avq = [None]
            pend = []  # (et_tile, col_off, goff, s) awaiting AV

            def alloc():
                avq[0] = psum.tile([128, 4, 65], F32, tag="avq", bufs=2,
                                   name=f"avq{g}_{hh}")

            def score_off(s):
                # off-diagonal key block, full query width
                def unit():
                    sp = psum.tile([128, TCH], F32, tag="mm", bufs=4,
                                   name=f"psS{g}_{hh}_{s}")
                    nc.tensor.matmul(
                        sp,
                        lhsT=kT[hs, bidx * T + s * 128:
                                bidx * T + (s + 1) * 128],
                        rhs=qT[hs, bidx * T + cc * TCH:
                               bidx * T + (cc + 1) * TCH],
                        start=True, stop=True,
                    )
                    et = epool.tile([128, TCH], BF16, tag="et",
                                    name=f"et{g}_{hh}_{s}")
                    nc.scalar.activation(et, sp, AF.Exp, scale=SCALE)
                    pend.append((et, 0, 0, s))
                return unit

            def score_diag(s):
                def unit():
                    j = s - 4 * cc
                    goff = 128 * j
                    w = TCH - goff
                    sp = psum.tile([128, TCH], F32, tag="mm", bufs=4,
                                   name=f"psS{g}_{hh}_{s}")
                    nc.tensor.matmul(
                        sp[:, 0:w],
                        lhsT=kT[hs, bidx * T + s * 128:
                                bidx * T + (s + 1) * 128],
                        rhs=qT[hs, bidx * T + cc * TCH + goff:
                               bidx * T + (cc + 1) * TCH],
                        start=True, stop=True,
                    )
                    et = epool.tile([128, TCH], BF16, tag="et",
                                    name=f"et{g}_{hh}_{s}")
                    nc.scalar.activation(et[:, 0:w], sp[:, 0:w], AF.Exp,
                                         scale=SCALE)
                    # zero the masked triangle (q < key) of the first 128
                    # computed columns, on the idle GPSIMD engine
                    nc.gpsimd.tensor_mul(et[:, 0:128], et[:, 0:128], tri_sb)
                    pend.append((et, 0, goff, s))
                return unit

            def av(idx):
                def unit():
                    et, coff, goff, s = pend[idx]
                    jlo = goff // 128
                    for j in range(jlo, 4):
                        nc.tensor.matmul(
                            avq[0][:, j, :],
                            lhsT=et[:, coff + j * 128 - goff:
                                    coff + (j + 1) * 128 - goff],
                            rhs=v1[:, bidx, s, hh, :],
                            start=(s == 0 and j == 0),
                            stop=(s == smax and j == 3),
                        )
                return unit

            def norm(j):
                def unit():
                    r = rpool.tile([128, 1], F32, tag="r",
                                   name=f"r{g}_{hh}_{j}")
                    nc.vector.reciprocal(r, avq[0][:, j, 64:65])
                    if g in (6, 4) and j % 2 == 0:
                        nc.scalar.mul(ao_nat[j][:, hh, :],
                                      avq[0][:, j, 0:64], r)
                    else:
                        nc.vector.tensor_scalar_mul(
                            ao_nat[j][:, hh, :], avq[0][:, j, 0:64], r)
                return unit

            return alloc, score_off, score_diag, av, norm

        h_ctx = [head_work(0), head_work(1)]
        units.append(h_ctx[0][0])
        units.append(h_ctx[1][0])
        for j in range(4):
            units.append(alloc_ao(j))
        # off-diagonal pairs then diagonal blocks; per-head streams
        # interleaved with one pending-AV of lag for pipelining
        navail = [0, 0]   # pend entries produced per head
        ndone = [0, 0]    # AV units emitted per head

        def emit_av_upto(hh, upto):
            while ndone[hh] < upto:
                units.append(h_ctx[hh][3](ndone[hh]))
                ndone[hh] += 1

        for s in range(0, 4 * cc):
            for hh in (0, 1):
                units.append(h_ctx[hh][1](s))
                navail[hh] += 1
                emit_av_upto(hh, navail[hh] - 1)
        for s in range(4 * cc, smax + 1):
            for hh in (0, 1):
                units.append(h_ctx[hh][2](s))
                navail[hh] += 1
                emit_av_upto(hh, navail[hh] - 1)
        for hh in (0, 1):
            emit_av_upto(hh, navail[hh])
        for j in range(4):
            for hh in (0, 1):
                units.append(h_ctx[hh][4](j))

        # transpose ao_nat -> aoT (phase C layout); deferred one round so
        # PE doesn't stall on the DVE norm-evict chain at chunk end
        def transp(j):
            def unit():
                tp = psum.tile([128, 128], BF16, tag="aux", bufs=2,
                               name=f"tp{g}_{j}")
                nc.tensor.transpose(tp, ao_nat[j], id_sb)
                nc.vector.tensor_copy(
                    aoT[:, g * TCH + j * 128:g * TCH + (j + 1) * 128], tp)
            return unit
        tails = [transp(j) for j in range(4)]
        return units, tails

    # ---------------- phase C units (output projection of one chunk) -------
    def make_c_units(g, last):
        units = []
        ysb = [None]

        def alloc():
            ysb[0] = ypool.tile([128, NE, TCH], BF16, tag="ysb",
                                name=f"ysb{g}")
        units.append(alloc)

        def oc_unit(oc):
            def unit():
                ps = psum.tile([128, TCH], F32, tag="mm", bufs=4,
                               name=f"psC{g}_{oc}")
                nc.tensor.matmul(
                    ps,
                    lhsT=wp_sb[:, oc * 128:(oc + 1) * 128],
                    rhs=aoT[:, g * TCH:(g + 1) * TCH],
                    start=True, stop=True,
                )
                if last and oc % 2 == 0:
                    nc.scalar.activation(ysb[0][:, oc, :], ps, AF.Identity,
                                         bias=bp_sb[:, oc:oc + 1])
                else:
                    nc.vector.tensor_scalar_add(ysb[0][:, oc, :], ps,
                                                bp_sb[:, oc:oc + 1])
            return unit
        for oc in range(NE):
            units.append(oc_unit(oc))

        def store(olo, ohi):
            def unit():
                nc.sync.dma_start(
                    out=yT.rearrange("(o p) t -> p o t", p=128)[
                        :, olo:ohi, g * TCH:(g + 1) * TCH],
                    in_=ysb[0][:, olo:ohi, :])
            return unit
        units.append(store(0, 4))
        units.append(store(4, NE))
        return units

    # ---------------- schedule -------------------------------------------
    def interleave(*streams):
        streams = [list(s) for s in streams if s]
        total = sum(len(s) for s in streams)
        out = []
        idx = [0] * len(streams)
        for _ in range(total):
            best, bestv = None, -1.0
            for i, s in enumerate(streams):
                rem = len(s) - idx[i]
                if rem <= 0:
                    continue
                frac = rem / len(s)
                if frac > bestv:
                    best, bestv = i, frac
            out.append(streams[best][idx[best]])
            idx[best] += 1
        return out

    # rounds: (A-chunks, B-chunk or None, TC-chunk or None)
    # batch 1's B chunks run in order c1, c3, c2, c0 so ACT-heavy chunks
    # get projection filler and the tail rounds are light.
    SCHED = [
        ([0, 1], 0, None),
        ([2],    1, None),
        ([3],    2, 0),
        ([4, 5], 3, 1),
        ([6, 7], 5, 2),
        ([],     7, 3),
        ([],     6, 5),
    ]
    b_tails = {}
    emitted_wp = False
    for rnd, (a_list, b_chunk, tc_chunk) in enumerate(SCHED):
        streams = []
        b_stream = None
        if b_chunk is not None:
            b_units, tails = make_b_units(b_chunk)
            b_tails[b_chunk] = tails
            b_stream = b_units
        if rnd == 0:
            # B0 reads the full qT/kT chunk 0: keep it strictly after A0
            streams.append(make_a_units(a_list[0]) + (b_stream or []))
            for a in a_list[1:]:
                streams.append(make_a_units(a))
        else:
            for a in a_list:
                streams.append(make_a_units(a))
            if not emitted_wp and b_chunk is not None and b_chunk >= 1:
                streams.append([load_wp])
                emitted_wp = True
            if b_stream is not None:
                streams.append(b_stream)
        if tc_chunk is not None:
            streams.append(b_tails.pop(tc_chunk) + make_c_units(tc_chunk, False))
        for u in interleave(*streams):
            u()

    # final round: B4 (light) + its own TC, with TC7 and TC6 as parallel
    # streams so the tail drains across all engines at once
    b4_units, b4_tails = make_b_units(4)
    final_streams = [
        b4_units + b4_tails + make_c_units(4, True),
        b_tails.pop(7) + make_c_units(7, True),
        b_tails.pop(6) + make_c_units(6, True),
    ]
    for u in interleave(*final_streams):
        u()


def build_bass():
    nc = bacc.Bacc("TRN2", target_bir_lowering=False, debug=False,
                   enable_asserts=False, num_devices=NCORES)
    xT = nc.dram_tensor("xT", [E, BT], BF16, kind="ExternalInput").ap()
    wqk = nc.dram_tensor("wqk", [E, 2 * 128], BF16, kind="ExternalInput").ap()
    wv = nc.dram_tensor("wv", [E, 128], BF16, kind="ExternalInput").ap()
    bqk = nc.dram_tensor("bqk", [2 * 128], F32, kind="ExternalInput").ap()
    bv = nc.dram_tensor("bv", [1, 128], BF16, kind="ExternalInput").ap()
    wp = nc.dram_tensor("wp", [128, E], BF16, kind="ExternalInput").ap()
    bp = nc.dram_tensor("bp", [E], F32, kind="ExternalInput").ap()
    trid = nc.dram_tensor("trid", [128, 128], BF16, kind="ExternalInput").ap()
    identd = nc.dram_tensor("identd", [128, 128], BF16,
                            kind="ExternalInput").ap()
    yT = nc.dram_tensor("yT", [E, BT], BF16, kind="ExternalOutput").ap()
    with tile.TileContext(nc) as tc:
        with nc.allow_low_precision(reason="bf16 operand production"):
            with ExitStack() as ctx:
                tc._ctx = ctx
                _emit(tc, yT, xT, wqk, wv, bqk, bv, wp, bp, trid, identd)
    nc.compile()
    return nc


def make_in_maps(inputs):
    stacked = np.asarray(inputs["stacked"], dtype=np.float32)
    w_attn = np.asarray(inputs["w_attn"], dtype=np.float32)
    b_attn = np.asarray(inputs["b_attn"], dtype=np.float32)
    w_proj = np.asarray(inputs["w_proj"], dtype=np.float32)
    b_proj = np.asarray(inputs["b_proj"], dtype=np.float32)

    bf = ml_dtypes.bfloat16
    xT = np.ascontiguousarray(stacked.reshape(BT, E).T).astype(bf)
    # keep-mask (1.0 where local query >= local key) for diagonal triangles
    rr = np.arange(128)
    tri = (rr[None, :] >= rr[:, None]).astype(np.float32).astype(bf)
    ident = np.eye(128, dtype=np.float32).astype(bf)

    in_maps = []
    for c in range(NCORES):
        lo = c * HPC * DH
        hi = lo + HPC * DH
        wqk_c = np.concatenate(
            [w_attn[:, lo:hi], w_attn[:, E + lo:E + hi]], axis=1)
        wv_c = w_attn[:, 2 * E + lo:2 * E + hi]
        bqk_c = np.concatenate([b_attn[lo:hi], b_attn[E + lo:E + hi]])
        bv_c = b_attn[2 * E + lo:2 * E + hi].reshape(1, 128)
        in_maps.append({
            "xT": xT,
            "wqk": np.ascontiguousarray(wqk_c).astype(bf),
            "wv": np.ascontiguousarray(wv_c).astype(bf),
            "bqk": np.ascontiguousarray(bqk_c),
            "bv": np.ascontiguousarray(bv_c).astype(bf),
            "wp": np.ascontiguousarray(w_proj[lo:hi, :]).astype(bf),
            "bp": b_proj if c == 0 else np.zeros_like(b_proj),
            "trid": tri,
            "identd": ident,
        })
    return in_maps


_NC = None


def _get_nc():
    global _NC
    if _NC is None:
        _NC = build_bass()
    return _NC


def run(inputs, trace=False):
    nc = _get_nc()
    in_maps = make_in_maps(inputs)
    res = bass_utils.run_bass_kernel_spmd(
        nc, in_maps, core_ids=list(range(NCORES)), trace=trace)
    acc = np.zeros((E, BT), dtype=np.float32)
    for out_map in res.results:
        acc += np.asarray(out_map["yT"]).astype(np.float32)
    y = np.ascontiguousarray(acc.T).reshape(B, T, E).astype(np.float32)
    return y, res


def kernel(**inputs):
    y, _ = run(inputs)
    return y


# revision 17
# speedup vs baseline: 1.0147x; 1.0147x over previous
"""Causal self-attention (GPT-2 style) on 8 Trainium2 NeuronCores.

Sharding: tensor-parallel over heads. Each of the 8 cores owns 2 of the 16
heads: it computes q/k/v projections for its heads (column-sharded w_attn),
runs causal attention for them, and multiplies by its row-slice of w_proj,
producing a partial (E, B*T) output in bf16. The host sums the 8 partials.

Kernel structure (v6):
- All matmul operands are bf16 (full PE rate at any moving size, half the
  DMA bytes). PSUM accumulation stays fp32.
- V is produced directly in natural [token, dh] layout (lhsT = x chunk,
  rhs = w_v): no transposes/copies for V. The projection bias is folded in
  as a rank-1 ones @ bias matmul in the same accumulation group.
- Scores are key-major S^T = K Q^T ([key, query] tiles) with causal
  narrowing: a diagonal key block only computes the queries that can see it
  (widths 512/384/256/128).
- Off-diagonal score blocks are computed in PAIRS into [128,1024] PSUM
  tiles (two banks, one accumulation group per bank) so a single exp
  instruction covers both blocks (halves the per-instruction ACT overhead).
- AV is computed transposed: out[q, d] = et[:, q-subtile]^T @ V with
  queries on PSUM partitions. The softmax denominator (ones-column of V)
  is then per-PARTITION: normalization is reciprocal([128,1]) +
  tensor_scalar_mul. The four per-quarter accumulation groups share one
  PSUM bank; start is set on the bank's first matmul and stop on its last
  (HW zero-region semantics: start wipes the whole 2KB bank).
- Causal masking is a 0/1 upper-triangular multiply on the otherwise-idle
  GPSIMD engine applied to the 128-wide diagonal triangle of et (exp
  without max-subtraction is safe here; masked lanes become exactly 0).
- attn-out is transposed back ([128,128] bf16 PE transposes, deferred one
  round so PE never waits on the DVE norm-evict chain) into the [dh,
  token] layout phase C needs.
- Phase C partials are staged per 512-token chunk in SBUF (bf16) and
  stored with two DMAs per chunk.
- Emission follows an explicit round schedule that pairs ACT-heavy
  attention chunks (large causal prefixes) with PE-heavy projection
  chunks, processing batch 1's chunks in the order c1, c3, c2, c0 so the
  tail rounds are light.
"""

import numpy as np
from contextlib import ExitStack

import ml_dtypes
import concourse.bass as bass
import concourse.bacc as bacc
import concourse.mybir as mybir
import concourse.tile as tile
from concourse import bass_utils

F32 = mybir.dt.float32
BF16 = mybir.dt.bfloat16
AF = mybir.ActivationFunctionType

B, T, E = 2, 2048, 1024
NH, DH = 16, 64
NCORES = 8
HPC = NH // NCORES          # heads per core = 2
BT = B * T                  # 4096 tokens total
TCH = 512                   # token chunk
NTC = BT // TCH             # 8 token chunks
NE = E // 128               # 8 contraction tiles over E
ST = T // 128               # 16 key tiles per batch
CPB = T // TCH              # 4 query chunks per batch
SCALE = 1.0 / 8.0           # 1/sqrt(DH)


def _emit(tc, yT, xT, wqk, wv, bqk, bv, wp, bp, trid, identd):
    nc = tc.nc
    ctx = tc._ctx

    singles = ctx.enter_context(tc.tile_pool(name="singles", bufs=1))
    xpool = ctx.enter_context(tc.tile_pool(name="xpool", bufs=3))
    epool = ctx.enter_context(tc.tile_pool(name="epool", bufs=8))
    apool = ctx.enter_context(tc.tile_pool(name="apool", bufs=12))
    rpool = ctx.enter_context(tc.tile_pool(name="rpool", bufs=8))
    ypool = ctx.enter_context(tc.tile_pool(name="ypool", bufs=2))
    psum = ctx.enter_context(tc.tile_pool(name="psum", space="PSUM", bufs=2))

    # --- persistent tiles; only wqk is loaded up front (x0 races it) ---
    wqk_sb = singles.tile([128, NE, 2 * 128], BF16)
    nc.scalar.dma_start(out=wqk_sb,
                        in_=wqk.rearrange("(e p) m -> p e m", p=128))
    wv_sb = singles.tile([128, NE, 128], BF16)
    bqk_sb = singles.tile([128, 2], F32)
    bv_sb = singles.tile([1, 128], BF16)
    bp_sb = singles.tile([128, NE], F32)
    tri_sb = singles.tile([128, 128], BF16)
    id_sb = singles.tile([128, 128], BF16)
    wp_sb = singles.tile([128, E], BF16)
    ones_sb = singles.tile([1, 128], BF16)
    nc.gpsimd.memset(ones_sb, 1.0)

    def load_consts():
        nc.scalar.dma_start(out=wv_sb,
                            in_=wv.rearrange("(e p) m -> p e m", p=128))
        nc.scalar.dma_start(out=bqk_sb,
                            in_=bqk.rearrange("(c p) -> p c", p=128))
        nc.scalar.dma_start(out=bv_sb, in_=bv)
        nc.scalar.dma_start(out=tri_sb, in_=trid)
        nc.scalar.dma_start(out=id_sb, in_=identd)

    def load_wp():
        nc.scalar.dma_start(out=wp_sb, in_=wp)
        nc.scalar.dma_start(out=bp_sb, in_=bp.rearrange("(c p) -> p c", p=128))

    qT = singles.tile([128, BT], BF16)   # rows: 2 heads x 64 dh
    kT = singles.tile([128, BT], BF16)
    aoT = singles.tile([128, BT], BF16)  # normalized attn-out^T
    # V natural layout per (batch, s-tile, head): [s-tok, dh] + ones col 64
    v1 = singles.tile([128, B, ST, HPC, 65], BF16)
    nc.gpsimd.memset(v1[:, :, :, :, 64:65], 1.0)

    xchunks = [None] * NTC

    # ---------------- phase A units (projection of one 512-token chunk) ----
    def make_a_units(g):
        units = []
        halves = 2 if g == 0 else 1

        def load_x(lo, hi):
            def unit():
                if xchunks[g] is None:
                    xchunks[g] = xpool.tile([128, NE, TCH], BF16, tag="xch",
                                            name=f"xch{g}")
                nc.sync.dma_start(
                    out=xchunks[g][:, :, lo:hi],
                    in_=xT.rearrange("(e p) t -> p e t", p=128)[
                        :, :, g * TCH + lo:g * TCH + hi],
                )
            return unit

        def qk(m, lo, hi):
            def unit():
                xch = xchunks[g]
                ps = psum.tile([128, hi - lo], F32, tag="mm", bufs=4,
                               name=f"psA{g}_{m}_{lo}")
                for e in range(NE):
                    nc.tensor.matmul(
                        ps,
                        lhsT=wqk_sb[:, e, m * 128:(m + 1) * 128],
                        rhs=xch[:, e, lo:hi],
                        start=(e == 0),
                        stop=(e == NE - 1),
                    )
                dst = qT if m == 0 else kT
                sl = dst[:, g * TCH + lo:g * TCH + hi]
                if m == 0:
                    nc.scalar.activation(sl, ps, AF.Identity,
                                         bias=bqk_sb[:, 0:1])
                else:
                    nc.vector.tensor_scalar_add(sl, ps, bqk_sb[:, 1:2])
            return unit

        bidx, cc = divmod(g, CPB)

        def vtile(stl):
            def unit():
                xch = xchunks[g]
                s_idx = cc * 4 + stl
                ps = psum.tile([128, 2, 64], F32, tag="aux", bufs=2,
                               name=f"psV{g}_{stl}")
                for e in range(NE):
                    nc.tensor.matmul(
                        ps,
                        lhsT=xch[:, e, stl * 128:(stl + 1) * 128],
                        rhs=wv_sb[:, e, :],
                        start=(e == 0), stop=False,
                    )
                # += ones^T (1x128) @ bv (1x128): broadcast bias over tokens
                nc.tensor.matmul(ps, lhsT=ones_sb, rhs=bv_sb,
                                 start=False, stop=True)
                if stl % 2 == 0:
                    nc.scalar.copy(v1[:, bidx, s_idx, :, 0:64], ps)
                else:
                    nc.vector.tensor_copy(v1[:, bidx, s_idx, :, 0:64], ps)
            return unit

        if halves == 2:
            units.append(load_x(0, 256))
            units.append(load_consts)
            units.append(qk(0, 0, 256))
            units.append(load_x(256, 512))
            units.append(qk(1, 0, 256))
            units.append(vtile(0))
            units.append(vtile(1))
            units.append(qk(0, 256, 512))
            units.append(qk(1, 256, 512))
            units.append(vtile(2))
            units.append(vtile(3))
        else:
            units.append(load_x(0, 512))
            units.append(qk(0, 0, 512))
            units.append(qk(1, 0, 512))
            for stl in range(4):
                units.append(vtile(stl))
        return units

    # ---------------- phase B units (attention for one chunk) --------------
    def make_b_units(g):
        bidx, cc = divmod(g, CPB)
        units = []
        smax = 4 * cc + 3
        ao_nat = [None] * 4  # per q-subtile [128, HPC, 64] bf16

        def alloc_ao(j):
            def unit():
                ao_nat[j] = apool.tile([128, HPC, 64], BF16, tag="ao",
                                       name=f"ao{g}_{j}")
            return unit

        def head_work(hh):
            hs = slice(hh * 64, (hh + 1) * 64)
            avq = [None]
            pend = []  # (et_tile, col_off, goff, s) awaiting AV

            def alloc():
                avq[0] = psum.tile([128, 4, 65], F32, tag="avq", bufs=2,
                                   name=f"avq{g}_{hh}")

            def score_off(s):
                # off-diagonal key block, full query width
                def unit():
                    sp = psum.tile([128, TCH], F32, tag="mm", bufs=4,
                                   name=f"psS{g}_{hh}_{s}")
                    nc.tensor.matmul(
                        sp,
                        lhsT=kT[hs, bidx * T + s * 128:
                                bidx * T + (s + 1) * 128],
                        rhs=qT[hs, bidx * T + cc * TCH:
                               bidx * T + (cc + 1) * TCH],
                        start=True, stop=True,
                    )
                    et = epool.tile([128, TCH], BF16, tag="et",
                                    name=f"et{g}_{hh}_{s}")
                    nc.scalar.activation(et, sp, AF.Exp, scale=SCALE)
                    pend.append((et, 0, 0, s))
                return unit

            def score_diag(s):
                def unit():
                    j = s - 4 * cc
                    goff = 128 * j
                    w = TCH - goff
                    sp = psum.tile([128, TCH], F32, tag="mm", bufs=4,
                                   name=f"psS{g}_{hh}_{s}")
                    nc.tensor.matmul(
                        sp[:, 0:w],
                        lhsT=kT[hs, bidx * T + s * 128:
                                bidx * T + (s + 1) * 128],
                        rhs=qT[hs, bidx * T + cc * TCH + goff:
                               bidx * T + (cc + 1) * TCH],
                        start=True, stop=True,
                    )
                    et = epool.tile([128, TCH], BF16, tag="et",
                                    name=f"et{g}_{hh}_{s}")
                    nc.scalar.activation(et[:, 0:w], sp[:, 0:w], AF.Exp,
                                         scale=SCALE)
                    # zero the masked triangle (q < key) of the first 128
                    # computed columns, on the idle GPSIMD engine
                    nc.gpsimd.tensor_mul(et[:, 0:128], et[:, 0:128], tri_sb)
                    pend.append((et, 0, goff, s))
                return unit

            def av(idx):
                def unit():
                    et, coff, goff, s = pend[idx]
                    jlo = goff // 128
                    for j in range(jlo, 4):
                        nc.tensor.matmul(
                            avq[0][:, j, :],
                            lhsT=et[:, coff + j * 128 - goff:
                                    coff + (j + 1) * 128 - goff],
                            rhs=v1[:, bidx, s, hh, :],
                            start=(s == 0 and j == 0),
                            stop=(s == smax and j == 3),
                        )
                return unit

            def norm(j):
                def unit():
                    r = rpool.tile([128, 1], F32, tag="r",
                                   name=f"r{g}_{hh}_{j}")
                    nc.vector.reciprocal(r, avq[0][:, j, 64:65])
                    if g in (6, 4) and j % 2 == 0:
                        nc.scalar.mul(ao_nat[j][:, hh, :],
                                      avq[0][:, j, 0:64], r)
                    else:
                        nc.vector.tensor_scalar_mul(
                            ao_nat[j][:, hh, :], avq[0][:, j, 0:64], r)
                return unit

            return alloc, score_off, score_diag, av, norm

        h_ctx = [head_work(0), head_work(1)]
        units.append(h_ctx[0][0])
        units.append(h_ctx[1][0])
        for j in range(4):
            units.append(alloc_ao(j))
        # off-diagonal pairs then diagonal blocks; per-head streams
        # interleaved with one pending-AV of lag for pipelining
        navail = [0, 0]   # pend entries produced per head
        ndone = [0, 0]    # AV units emitted per head

        def emit_av_upto(hh, upto):
            while ndone[hh] < upto:
                units.append(h_ctx[hh][3](ndone[hh]))
                ndone[hh] += 1

        for s in range(0, 4 * cc):
            for hh in (0, 1):
                units.append(h_ctx[hh][1](s))
                navail[hh] += 1
                emit_av_upto(hh, navail[hh] - 1)
        for s in range(4 * cc, smax + 1):
            for hh in (0, 1):
                units.append(h_ctx[hh][2](s))
                navail[hh] += 1
                emit_av_upto(hh, navail[hh] - 1)
        for hh in (0, 1):
            emit_av_upto(hh, navail[hh])
        for j in range(4):
            for hh in (0, 1):
                units.append(h_ctx[hh][4](j))

        # transpose ao_nat -> aoT (phase C layout); deferred one round so
        # PE doesn't stall on the DVE norm-evict chain at chunk end
        def transp(j):
            def unit():
                tp = psum.tile([128, 128], BF16, tag="aux", bufs=2,
                               name=f"tp{g}_{j}")
                nc.tensor.transpose(tp, ao_nat[j], id_sb)
                nc.vector.tensor_copy(
                    aoT[:, g * TCH + j * 128:g * TCH + (j + 1) * 128], tp)
            return unit
        tails = [transp(j) for j in range(4)]
        return units, tails

    # ---------------- phase C units (output projection of one chunk) -------
    def make_c_units(g, last):
        units = []
        ysb = [None]

        def alloc():
            ysb[0] = ypool.tile([128, NE, TCH], BF16, tag="ysb",
                                name=f"ysb{g}")
        units.append(alloc)

        def oc_unit(oc):
            def unit():
                ps = psum.tile([128, TCH], F32, tag="mm", bufs=4,
                               name=f"psC{g}_{oc}")
                nc.tensor.matmul(
                    ps,
                    lhsT=wp_sb[:, oc * 128:(oc + 1) * 128],
                    rhs=aoT[:, g * TCH:(g + 1) * TCH],
                    start=True, stop=True,
                )
                if last and oc % 2 == 0:
                    nc.scalar.activation(ysb[0][:, oc, :], ps, AF.Identity,
                                         bias=bp_sb[:, oc:oc + 1])
                else:
                    nc.vector.tensor_scalar_add(ysb[0][:, oc, :], ps,
                                                bp_sb[:, oc:oc + 1])
            return unit
        for oc in range(NE):
            units.append(oc_unit(oc))

        def store(olo, ohi):
            def unit():
                nc.sync.dma_start(
                    out=yT.rearrange("(o p) t -> p o t", p=128)[
                        :, olo:ohi, g * TCH:(g + 1) * TCH],
                    in_=ysb[0][:, olo:ohi, :])
            return unit
        units.append(store(0, 4))
        units.append(store(4, NE))
        return units

    # ---------------- schedule -------------------------------------------
    def interleave(*streams):
        streams = [list(s) for s in streams if s]
        total = sum(len(s) for s in streams)
        out = []
        idx = [0] * len(streams)
        for _ in range(total):
            best, bestv = None, -1.0
            for i, s in enumerate(streams):
                rem = len(s) - idx[i]
                if rem <= 0:
                    continue
                frac = rem / len(s)
                if frac > bestv:
                    best, bestv = i, frac
            out.append(streams[best][idx[best]])
            idx[best] += 1
        return out

    # rounds: (A-chunks, B-chunk or None, TC-chunk or None)
    # batch 1's B chunks run in order c1, c3, c2, c0 so ACT-heavy chunks
    # get projection filler and the tail rounds are light.
    SCHED = [
        ([0, 1], 0, None),
        ([2],    1, None),
        ([3],    2, 0),
        ([4, 5], 3, 1),
        ([6, 7], 5, 2),
        ([],     7, 3),
        ([],     6, 5),
    ]
    b_tails = {}
    emitted_wp = False
    for rnd, (a_list, b_chunk, tc_chunk) in enumerate(SCHED):
        streams = []
        b_stream = None
        if b_chunk is not None:
            b_units, tails = make_b_units(b_chunk)
            b_tails[b_chunk] = tails
            b_stream = b_units
        if rnd == 0:
            # B0 reads the full qT/kT chunk 0: keep it strictly after A0
            streams.append(make_a_units(a_list[0]) + (b_stream or []))
            for a in a_list[1:]:
                streams.append(make_a_units(a))
        else:
            for a in a_list:
                streams.append(make_a_units(a))
            if not emitted_wp and b_chunk is not None and b_chunk >= 1:
                streams.append([load_wp])
                emitted_wp = True
            if b_stream is not None:
                streams.append(b_stream)
        if tc_chunk is not None:
            streams.append(b_tails.pop(tc_chunk) + make_c_units(tc_chunk, False))
        for u in interleave(*streams):
            u()

    # final rounds: B4's scores/AV (deps long satisfied) run alongside the
    # TC streams of chunks 7 and 6; B4's norm chain + its own TC go last so
    # slow-dependency units never block ready work in the in-order queues
    b4_units, b4_tails = make_b_units(4)
    b4_main, b4_norm = b4_units[:-8], b4_units[-8:]
    for u in interleave(
        b4_main,
        b_tails.pop(7) + make_c_units(7, True),
        b_tails.pop(6) + make_c_units(6, True),
    ):
        u()
    for u in b4_norm + b4_tails + make_c_units(4, True):
        u()


def build_bass():
    nc = bacc.Bacc("TRN2", target_bir_lowering=False, debug=False,
                   enable_asserts=False, num_devices=NCORES)
    xT = nc.dram_tensor("xT", [E, BT], BF16, kind="ExternalInput").ap()
    wqk = nc.dram_tensor("wqk", [E, 2 * 128], BF16, kind="ExternalInput").ap()
    wv = nc.dram_tensor("wv", [E, 128], BF16, kind="ExternalInput").ap()
    bqk = nc.dram_tensor("bqk", [2 * 128], F32, kind="ExternalInput").ap()
    bv = nc.dram_tensor("bv", [1, 128], BF16, kind="ExternalInput").ap()
    wp = nc.dram_tensor("wp", [128, E], BF16, kind="ExternalInput").ap()
    bp = nc.dram_tensor("bp", [E], F32, kind="ExternalInput").ap()
    trid = nc.dram_tensor("trid", [128, 128], BF16, kind="ExternalInput").ap()
    identd = nc.dram_tensor("identd", [128, 128], BF16,
                            kind="ExternalInput").ap()
    yT = nc.dram_tensor("yT", [E, BT], BF16, kind="ExternalOutput").ap()
    with tile.TileContext(nc) as tc:
        with nc.allow_low_precision(reason="bf16 operand production"):
            with ExitStack() as ctx:
                tc._ctx = ctx
                _emit(tc, yT, xT, wqk, wv, bqk, bv, wp, bp, trid, identd)
    nc.compile()
    return nc


def make_in_maps(inputs):
    stacked = np.asarray(inputs["stacked"], dtype=np.float32)
    w_attn = np.asarray(inputs["w_attn"], dtype=np.float32)
    b_attn = np.asarray(inputs["b_attn"], dtype=np.float32)
    w_proj = np.asarray(inputs["w_proj"], dtype=np.float32)
    b_proj = np.asarray(inputs["b_proj"], dtype=np.float32)

    bf = ml_dtypes.bfloat16
    xT = np.ascontiguousarray(stacked.reshape(BT, E).T).astype(bf)
    # keep-mask (1.0 where local query >= local key) for diagonal triangles
    rr = np.arange(128)
    tri = (rr[None, :] >= rr[:, None]).astype(np.float32).astype(bf)
    ident = np.eye(128, dtype=np.float32).astype(bf)

    in_maps = []
    for c in range(NCORES):
        lo = c * HPC * DH
        hi = lo + HPC * DH
        wqk_c = np.concatenate(
            [w_attn[:, lo:hi], w_attn[:, E + lo:E + hi]], axis=1)
        wv_c = w_attn[:, 2 * E + lo:2 * E + hi]
        bqk_c = np.concatenate([b_attn[lo:hi], b_attn[E + lo:E + hi]])
        bv_c = b_attn[2 * E + lo:2 * E + hi].reshape(1, 128)
        in_maps.append({
            "xT": xT,
            "wqk": np.ascontiguousarray(wqk_c).astype(bf),
            "wv": np.ascontiguousarray(wv_c).astype(bf),
            "bqk": np.ascontiguousarray(bqk_c),
            "bv": np.ascontiguousarray(bv_c).astype(bf),
            "wp": np.ascontiguousarray(w_proj[lo:hi, :]).astype(bf),
            "bp": b_proj if c == 0 else np.zeros_like(b_proj),
            "trid": tri,
            "identd": ident,
        })
    return in_maps


_NC = None


def _get_nc():
    global _NC
    if _NC is None:
        _NC = build_bass()
    return _NC


def run(inputs, trace=False):
    nc = _get_nc()
    in_maps = make_in_maps(inputs)
    res = bass_utils.run_bass_kernel_spmd(
        nc, in_maps, core_ids=list(range(NCORES)), trace=trace)
    acc = np.zeros((E, BT), dtype=np.float32)
    for out_map in res.results:
        acc += np.asarray(out_map["yT"]).astype(np.float32)
    y = np.ascontiguousarray(acc.T).reshape(B, T, E).astype(np.float32)
    return y, res


def kernel(**inputs):
    y, _ = run(inputs)
    return y


# revision 18
# speedup vs baseline: 1.0373x; 1.0223x over previous
"""Causal self-attention (GPT-2 style) on 8 Trainium2 NeuronCores.

Sharding: tensor-parallel over heads. Each of the 8 cores owns 2 of the 16
heads: it computes q/k/v projections for its heads (column-sharded w_attn),
runs causal attention for them, and multiplies by its row-slice of w_proj,
producing a partial (E, B*T) output in bf16. The host sums the 8 partials.

Kernel structure (v6):
- All matmul operands are bf16 (full PE rate at any moving size, half the
  DMA bytes). PSUM accumulation stays fp32.
- V is produced directly in natural [token, dh] layout (lhsT = x chunk,
  rhs = w_v): no transposes/copies for V. The projection bias is folded in
  as a rank-1 ones @ bias matmul in the same accumulation group.
- Scores are key-major S^T = K Q^T ([key, query] tiles) with causal
  narrowing: a diagonal key block only computes the queries that can see it
  (widths 512/384/256/128).
- Off-diagonal score blocks are computed in PAIRS into [128,1024] PSUM
  tiles (two banks, one accumulation group per bank) so a single exp
  instruction covers both blocks (halves the per-instruction ACT overhead).
- AV is computed transposed: out[q, d] = et[:, q-subtile]^T @ V with
  queries on PSUM partitions. The softmax denominator (ones-column of V)
  is then per-PARTITION: normalization is reciprocal([128,1]) +
  tensor_scalar_mul. The four per-quarter accumulation groups share one
  PSUM bank; start is set on the bank's first matmul and stop on its last
  (HW zero-region semantics: start wipes the whole 2KB bank).
- Causal masking is a 0/1 upper-triangular multiply on the otherwise-idle
  GPSIMD engine applied to the 128-wide diagonal triangle of et (exp
  without max-subtraction is safe here; masked lanes become exactly 0).
- attn-out is transposed back ([128,128] bf16 PE transposes, deferred one
  round so PE never waits on the DVE norm-evict chain) into the [dh,
  token] layout phase C needs.
- Phase C partials are staged per 512-token chunk in SBUF (bf16) and
  stored with two DMAs per chunk.
- Emission follows an explicit round schedule that pairs ACT-heavy
  attention chunks (large causal prefixes) with PE-heavy projection
  chunks, processing batch 1's chunks in the order c1, c3, c2, c0 so the
  tail rounds are light.
"""

import numpy as np
from contextlib import ExitStack

import ml_dtypes
import concourse.bass as bass
import concourse.bacc as bacc
import concourse.mybir as mybir
import concourse.tile as tile
from concourse import bass_utils

F32 = mybir.dt.float32
BF16 = mybir.dt.bfloat16
AF = mybir.ActivationFunctionType

B, T, E = 2, 2048, 1024
NH, DH = 16, 64
NCORES = 8
HPC = NH // NCORES          # heads per core = 2
BT = B * T                  # 4096 tokens total
TCH = 512                   # token chunk
NTC = BT // TCH             # 8 token chunks
NE = E // 128               # 8 contraction tiles over E
ST = T // 128               # 16 key tiles per batch
CPB = T // TCH              # 4 query chunks per batch
SCALE = 1.0 / 8.0           # 1/sqrt(DH)


def _emit(tc, yT, xT, wqk, wv, bqk, bv, wp, bp, trid, identd):
    nc = tc.nc
    ctx = tc._ctx

    singles = ctx.enter_context(tc.tile_pool(name="singles", bufs=1))
    xpool = ctx.enter_context(tc.tile_pool(name="xpool", bufs=3))
    epool = ctx.enter_context(tc.tile_pool(name="epool", bufs=8))
    apool = ctx.enter_context(tc.tile_pool(name="apool", bufs=12))
    rpool = ctx.enter_context(tc.tile_pool(name="rpool", bufs=8))
    ypool = ctx.enter_context(tc.tile_pool(name="ypool", bufs=2))
    psum = ctx.enter_context(tc.tile_pool(name="psum", space="PSUM", bufs=2))

    # --- persistent tiles; only wqk is loaded up front (x0 races it) ---
    wqk_sb = singles.tile([128, NE, 2 * 128], BF16)
    nc.scalar.dma_start(out=wqk_sb,
                        in_=wqk.rearrange("(e p) m -> p e m", p=128))
    wv_sb = singles.tile([128, NE, 128], BF16)
    bqk_sb = singles.tile([128, 2], F32)
    bv_sb = singles.tile([1, 128], BF16)
    bp_sb = singles.tile([128, NE], F32)
    tri_sb = singles.tile([128, 128], BF16)
    id_sb = singles.tile([128, 128], BF16)
    wp_sb = singles.tile([128, E], BF16)
    ones_sb = singles.tile([1, 128], BF16)
    nc.gpsimd.memset(ones_sb, 1.0)

    def load_consts():
        nc.scalar.dma_start(out=wv_sb,
                            in_=wv.rearrange("(e p) m -> p e m", p=128))
        nc.scalar.dma_start(out=bqk_sb,
                            in_=bqk.rearrange("(c p) -> p c", p=128))
        nc.scalar.dma_start(out=bv_sb, in_=bv)
        nc.scalar.dma_start(out=tri_sb, in_=trid)
        nc.scalar.dma_start(out=id_sb, in_=identd)

    def load_wp():
        nc.scalar.dma_start(out=wp_sb, in_=wp)
        nc.scalar.dma_start(out=bp_sb, in_=bp.rearrange("(c p) -> p c", p=128))

    qT = singles.tile([128, BT], BF16)   # rows: 2 heads x 64 dh
    kT = singles.tile([128, BT], BF16)
    aoT = singles.tile([128, BT], BF16)  # normalized attn-out^T
    # V natural layout per (batch, s-tile, head): [s-tok, dh] + ones col 64
    v1 = singles.tile([128, B, ST, HPC, 65], BF16)
    nc.gpsimd.memset(v1[:, :, :, :, 64:65], 1.0)

    xchunks = [None] * NTC

    # ---------------- phase A units (projection of one 512-token chunk) ----
    def make_a_units(g):
        units = []
        halves = 2 if g == 0 else 1

        def load_x(lo, hi):
            def unit():
                if xchunks[g] is None:
                    xchunks[g] = xpool.tile([128, NE, TCH], BF16, tag="xch",
                                            name=f"xch{g}")
                nc.sync.dma_start(
                    out=xchunks[g][:, :, lo:hi],
                    in_=xT.rearrange("(e p) t -> p e t", p=128)[
                        :, :, g * TCH + lo:g * TCH + hi],
                )
            return unit

        def qk(m, lo, hi):
            def unit():
                xch = xchunks[g]
                ps = psum.tile([128, hi - lo], F32, tag="mm", bufs=4,
                               name=f"psA{g}_{m}_{lo}")
                for e in range(NE):
                    nc.tensor.matmul(
                        ps,
                        lhsT=wqk_sb[:, e, m * 128:(m + 1) * 128],
                        rhs=xch[:, e, lo:hi],
                        start=(e == 0),
                        stop=(e == NE - 1),
                    )
                dst = qT if m == 0 else kT
                sl = dst[:, g * TCH + lo:g * TCH + hi]
                if m == 0:
                    nc.scalar.activation(sl, ps, AF.Identity,
                                         bias=bqk_sb[:, 0:1])
                else:
                    nc.vector.tensor_scalar_add(sl, ps, bqk_sb[:, 1:2])
            return unit

        bidx, cc = divmod(g, CPB)

        def vtile(stl):
            def unit():
                xch = xchunks[g]
                s_idx = cc * 4 + stl
                ps = psum.tile([128, 2, 64], F32, tag="aux", bufs=2,
                               name=f"psV{g}_{stl}")
                for e in range(NE):
                    nc.tensor.matmul(
                        ps,
                        lhsT=xch[:, e, stl * 128:(stl + 1) * 128],
                        rhs=wv_sb[:, e, :],
                        start=(e == 0), stop=False,
                    )
                # += ones^T (1x128) @ bv (1x128): broadcast bias over tokens
                nc.tensor.matmul(ps, lhsT=ones_sb, rhs=bv_sb,
                                 start=False, stop=True)
                if stl % 2 == 0:
                    nc.scalar.copy(v1[:, bidx, s_idx, :, 0:64], ps)
                else:
                    nc.vector.tensor_copy(v1[:, bidx, s_idx, :, 0:64], ps)
            return unit

        if halves == 2:
            units.append(load_x(0, 256))
            units.append(load_consts)
            units.append(qk(0, 0, 256))
            units.append(load_x(256, 512))
            units.append(qk(1, 0, 256))
            units.append(vtile(0))
            units.append(vtile(1))
            units.append(qk(0, 256, 512))
            units.append(qk(1, 256, 512))
            units.append(vtile(2))
            units.append(vtile(3))
        else:
            units.append(load_x(0, 512))
            units.append(qk(0, 0, 512))
            units.append(qk(1, 0, 512))
            for stl in range(4):
                units.append(vtile(stl))
        return units

    # ---------------- phase B units (attention for one chunk) --------------
    def make_b_units(g):
        bidx, cc = divmod(g, CPB)
        units = []
        smax = 4 * cc + 3
        ao_nat = [None] * 4  # per q-subtile [128, HPC, 64] bf16

        def alloc_ao(j):
            def unit():
                ao_nat[j] = apool.tile([128, HPC, 64], BF16, tag="ao",
                                       name=f"ao{g}_{j}")
            return unit

        def head_work(hh):
            hs = slice(hh * 64, (hh + 1) * 64)
            avq = [None]
            pend = []  # (et_tile, col_off, goff, s) awaiting AV

            def alloc():
                avq[0] = psum.tile([128, 4, 65], F32, tag="avq", bufs=2,
                                   name=f"avq{g}_{hh}")

            def score_off(s):
                # off-diagonal key block, full query width
                def unit():
                    sp = psum.tile([128, TCH], F32, tag="mm", bufs=4,
                                   name=f"psS{g}_{hh}_{s}")
                    nc.tensor.matmul(
                        sp,
                        lhsT=kT[hs, bidx * T + s * 128:
                                bidx * T + (s + 1) * 128],
                        rhs=qT[hs, bidx * T + cc * TCH:
                               bidx * T + (cc + 1) * TCH],
                        start=True, stop=True,
                    )
                    et = epool.tile([128, TCH], BF16, tag="et",
                                    name=f"et{g}_{hh}_{s}")
                    nc.scalar.activation(et, sp, AF.Exp, scale=SCALE)
                    pend.append((et, 0, 0, s))
                return unit

            def score_diag(s):
                def unit():
                    j = s - 4 * cc
                    goff = 128 * j
                    w = TCH - goff
                    sp = psum.tile([128, TCH], F32, tag="mm", bufs=4,
                                   name=f"psS{g}_{hh}_{s}")
                    nc.tensor.matmul(
                        sp[:, 0:w],
                        lhsT=kT[hs, bidx * T + s * 128:
                                bidx * T + (s + 1) * 128],
                        rhs=qT[hs, bidx * T + cc * TCH + goff:
                               bidx * T + (cc + 1) * TCH],
                        start=True, stop=True,
                    )
                    et = epool.tile([128, TCH], BF16, tag="et",
                                    name=f"et{g}_{hh}_{s}")
                    nc.scalar.activation(et[:, 0:w], sp[:, 0:w], AF.Exp,
                                         scale=SCALE)
                    # zero the masked triangle (q < key) of the first 128
                    # computed columns, on the idle GPSIMD engine
                    nc.gpsimd.tensor_mul(et[:, 0:128], et[:, 0:128], tri_sb)
                    pend.append((et, 0, goff, s))
                return unit

            def av(idx):
                def unit():
                    et, coff, goff, s = pend[idx]
                    jlo = goff // 128
                    for j in range(jlo, 4):
                        nc.tensor.matmul(
                            avq[0][:, j, :],
                            lhsT=et[:, coff + j * 128 - goff:
                                    coff + (j + 1) * 128 - goff],
                            rhs=v1[:, bidx, s, hh, :],
                            start=(s == 0 and j == 0),
                            stop=(s == smax and j == 3),
                        )
                return unit

            def norm(j):
                def unit():
                    r = rpool.tile([128, 1], F32, tag="r",
                                   name=f"r{g}_{hh}_{j}")
                    nc.vector.reciprocal(r, avq[0][:, j, 64:65])
                    if g in (6, 4) and j % 2 == 0:
                        nc.scalar.mul(ao_nat[j][:, hh, :],
                                      avq[0][:, j, 0:64], r)
                    else:
                        nc.vector.tensor_scalar_mul(
                            ao_nat[j][:, hh, :], avq[0][:, j, 0:64], r)
                return unit

            return alloc, score_off, score_diag, av, norm

        h_ctx = [head_work(0), head_work(1)]
        units.append(h_ctx[0][0])
        units.append(h_ctx[1][0])
        for j in range(4):
            units.append(alloc_ao(j))
        # off-diagonal pairs then diagonal blocks; per-head streams
        # interleaved with one pending-AV of lag for pipelining
        navail = [0, 0]   # pend entries produced per head
        ndone = [0, 0]    # AV units emitted per head

        def emit_av_upto(hh, upto):
            while ndone[hh] < upto:
                units.append(h_ctx[hh][3](ndone[hh]))
                ndone[hh] += 1

        for s in range(0, 4 * cc):
            for hh in (0, 1):
                units.append(h_ctx[hh][1](s))
                navail[hh] += 1
                emit_av_upto(hh, navail[hh] - 1)
        for s in range(4 * cc, smax + 1):
            for hh in (0, 1):
                units.append(h_ctx[hh][2](s))
                navail[hh] += 1
                emit_av_upto(hh, navail[hh] - 1)
        for hh in (0, 1):
            emit_av_upto(hh, navail[hh])
        for j in range(4):
            for hh in (0, 1):
                units.append(h_ctx[hh][4](j))

        # transpose ao_nat -> aoT (phase C layout); deferred one round so
        # PE doesn't stall on the DVE norm-evict chain at chunk end
        def transp(j):
            def unit():
                tp = psum.tile([128, 128], BF16, tag="aux", bufs=2,
                               name=f"tp{g}_{j}")
                nc.tensor.transpose(tp, ao_nat[j], id_sb)
                nc.vector.tensor_copy(
                    aoT[:, g * TCH + j * 128:g * TCH + (j + 1) * 128], tp)
            return unit
        tails = [transp(j) for j in range(4)]
        return units, tails

    # ---------------- phase C units (output projection of one chunk) -------
    def make_c_units(g, last):
        units = []
        ysb = [None]

        def alloc():
            ysb[0] = ypool.tile([128, NE, TCH], BF16, tag="ysb",
                                name=f"ysb{g}")
        units.append(alloc)

        def oc_unit(oc):
            def unit():
                ps = psum.tile([128, TCH], F32, tag="mm", bufs=4,
                               name=f"psC{g}_{oc}")
                nc.tensor.matmul(
                    ps,
                    lhsT=wp_sb[:, oc * 128:(oc + 1) * 128],
                    rhs=aoT[:, g * TCH:(g + 1) * TCH],
                    start=True, stop=True,
                )
                if last and oc % 2 == 0:
                    nc.scalar.activation(ysb[0][:, oc, :], ps, AF.Identity,
                                         bias=bp_sb[:, oc:oc + 1])
                else:
                    nc.vector.tensor_scalar_add(ysb[0][:, oc, :], ps,
                                                bp_sb[:, oc:oc + 1])
            return unit
        for oc in range(NE):
            units.append(oc_unit(oc))

        def store(olo, ohi):
            def unit():
                nc.sync.dma_start(
                    out=yT.rearrange("(o p) t -> p o t", p=128)[
                        :, olo:ohi, g * TCH:(g + 1) * TCH],
                    in_=ysb[0][:, olo:ohi, :])
            return unit
        units.append(store(0, 4))
        units.append(store(4, NE))
        return units

    # ---------------- schedule -------------------------------------------
    def interleave(*streams):
        streams = [list(s) for s in streams if s]
        total = sum(len(s) for s in streams)
        out = []
        idx = [0] * len(streams)
        for _ in range(total):
            best, bestv = None, -1.0
            for i, s in enumerate(streams):
                rem = len(s) - idx[i]
                if rem <= 0:
                    continue
                frac = rem / len(s)
                if frac > bestv:
                    best, bestv = i, frac
            out.append(streams[best][idx[best]])
            idx[best] += 1
        return out

    # rounds: (A-chunks, B-chunk or None, TC-chunk or None)
    # batch 1's B chunks run in order c1, c3, c2, c0 so ACT-heavy chunks
    # get projection filler and the tail rounds are light.
    SCHED = [
        ([0, 1], 0, None),
        ([2],    1, None),
        ([3],    2, 0),
        ([4, 5], 3, 1),
        ([6, 7], 5, 2),
        ([],     7, 3),
        ([],     6, 5),
    ]
    b_tails = {}
    emitted_wp = False
    for rnd, (a_list, b_chunk, tc_chunk) in enumerate(SCHED):
        streams = []
        b_stream = None
        if b_chunk is not None:
            b_units, tails = make_b_units(b_chunk)
            b_tails[b_chunk] = tails
            b_stream = b_units
        if rnd == 0:
            # B0 reads the full qT/kT chunk 0: keep it strictly after A0
            streams.append(make_a_units(a_list[0]) + (b_stream or []))
            for a in a_list[1:]:
                streams.append(make_a_units(a))
        else:
            for a in a_list:
                streams.append(make_a_units(a))
            if not emitted_wp and b_chunk is not None and b_chunk >= 1:
                streams.append([load_wp])
                emitted_wp = True
            if b_stream is not None:
                streams.append(b_stream)
        if tc_chunk is not None:
            streams.append(b_tails.pop(tc_chunk) + make_c_units(tc_chunk, False))
        for u in interleave(*streams):
            u()

    # tail rounds: keep per-chunk rounds (slow-dependency units at the
    # head of an in-order queue block ready work, so don't merge them)
    b4_units, b4_tails = make_b_units(4)
    for u in interleave(b4_units, b_tails.pop(7) + make_c_units(7, True)):
        u()
    for u in interleave(b_tails.pop(6) + make_c_units(6, True)):
        u()
    for u in interleave(b4_tails + make_c_units(4, True)):
        u()


def build_bass():
    nc = bacc.Bacc("TRN2", target_bir_lowering=False, debug=False,
                   enable_asserts=False, num_devices=NCORES)
    xT = nc.dram_tensor("xT", [E, BT], BF16, kind="ExternalInput").ap()
    wqk = nc.dram_tensor("wqk", [E, 2 * 128], BF16, kind="ExternalInput").ap()
    wv = nc.dram_tensor("wv", [E, 128], BF16, kind="ExternalInput").ap()
    bqk = nc.dram_tensor("bqk", [2 * 128], F32, kind="ExternalInput").ap()
    bv = nc.dram_tensor("bv", [1, 128], BF16, kind="ExternalInput").ap()
    wp = nc.dram_tensor("wp", [128, E], BF16, kind="ExternalInput").ap()
    bp = nc.dram_tensor("bp", [E], F32, kind="ExternalInput").ap()
    trid = nc.dram_tensor("trid", [128, 128], BF16, kind="ExternalInput").ap()
    identd = nc.dram_tensor("identd", [128, 128], BF16,
                            kind="ExternalInput").ap()
    yT = nc.dram_tensor("yT", [E, BT], BF16, kind="ExternalOutput").ap()
    with tile.TileContext(nc) as tc:
        with nc.allow_low_precision(reason="bf16 operand production"):
            with ExitStack() as ctx:
                tc._ctx = ctx
                _emit(tc, yT, xT, wqk, wv, bqk, bv, wp, bp, trid, identd)
    nc.compile()
    return nc


def make_in_maps(inputs):
    stacked = np.asarray(inputs["stacked"], dtype=np.float32)
    w_attn = np.asarray(inputs["w_attn"], dtype=np.float32)
    b_attn = np.asarray(inputs["b_attn"], dtype=np.float32)
    w_proj = np.asarray(inputs["w_proj"], dtype=np.float32)
    b_proj = np.asarray(inputs["b_proj"], dtype=np.float32)

    bf = ml_dtypes.bfloat16
    xT = np.ascontiguousarray(stacked.reshape(BT, E).T).astype(bf)
    # keep-mask (1.0 where local query >= local key) for diagonal triangles
    rr = np.arange(128)
    tri = (rr[None, :] >= rr[:, None]).astype(np.float32).astype(bf)
    ident = np.eye(128, dtype=np.float32).astype(bf)

    in_maps = []
    for c in range(NCORES):
        lo = c * HPC * DH
        hi = lo + HPC * DH
        wqk_c = np.concatenate(
            [w_attn[:, lo:hi], w_attn[:, E + lo:E + hi]], axis=1)
        wv_c = w_attn[:, 2 * E + lo:2 * E + hi]
        bqk_c = np.concatenate([b_attn[lo:hi], b_attn[E + lo:E + hi]])
        bv_c = b_attn[2 * E + lo:2 * E + hi].reshape(1, 128)
        in_maps.append({
            "xT": xT,
            "wqk": np.ascontiguousarray(wqk_c).astype(bf),
            "wv": np.ascontiguousarray(wv_c).astype(bf),
            "bqk": np.ascontiguousarray(bqk_c),
            "bv": np.ascontiguousarray(bv_c).astype(bf),
            "wp": np.ascontiguousarray(w_proj[lo:hi, :]).astype(bf),
            "bp": b_proj if c == 0 else np.zeros_like(b_proj),
            "trid": tri,
            "identd": ident,
        })
    return in_maps


_NC = None


def _get_nc():
    global _NC
    if _NC is None:
        _NC = build_bass()
    return _NC


def run(inputs, trace=False):
    nc = _get_nc()
    in_maps = make_in_maps(inputs)
    res = bass_utils.run_bass_kernel_spmd(
        nc, in_maps, core_ids=list(range(NCORES)), trace=trace)
    acc = np.zeros((E, BT), dtype=np.float32)
    for out_map in res.results:
        acc += np.asarray(out_map["yT"]).astype(np.float32)
    y = np.ascontiguousarray(acc.T).reshape(B, T, E).astype(np.float32)
    return y, res


def kernel(**inputs):
    y, _ = run(inputs)
    return y


# revision 19
# speedup vs baseline: 1.0591x; 1.0210x over previous
"""Causal self-attention (GPT-2 style) on 8 Trainium2 NeuronCores.

Sharding: tensor-parallel over heads. Each of the 8 cores owns 2 of the 16
heads: it computes q/k/v projections for its heads (column-sharded w_attn),
runs causal attention for them, and multiplies by its row-slice of w_proj,
producing a partial (E, B*T) output in bf16. The host sums the 8 partials.

Kernel structure (v6):
- All matmul operands are bf16 (full PE rate at any moving size, half the
  DMA bytes). PSUM accumulation stays fp32.
- V is produced directly in natural [token, dh] layout (lhsT = x chunk,
  rhs = w_v): no transposes/copies for V. The projection bias is folded in
  as a rank-1 ones @ bias matmul in the same accumulation group.
- Scores are key-major S^T = K Q^T ([key, query] tiles) with causal
  narrowing: a diagonal key block only computes the queries that can see it
  (widths 512/384/256/128).
- Off-diagonal score blocks are computed in PAIRS into [128,1024] PSUM
  tiles (two banks, one accumulation group per bank) so a single exp
  instruction covers both blocks (halves the per-instruction ACT overhead).
- AV is computed transposed: out[q, d] = et[:, q-subtile]^T @ V with
  queries on PSUM partitions. The softmax denominator (ones-column of V)
  is then per-PARTITION: normalization is reciprocal([128,1]) +
  tensor_scalar_mul. The four per-quarter accumulation groups share one
  PSUM bank; start is set on the bank's first matmul and stop on its last
  (HW zero-region semantics: start wipes the whole 2KB bank).
- Causal masking is a 0/1 upper-triangular multiply on the otherwise-idle
  GPSIMD engine applied to the 128-wide diagonal triangle of et (exp
  without max-subtraction is safe here; masked lanes become exactly 0).
- attn-out is transposed back ([128,128] bf16 PE transposes, deferred one
  round so PE never waits on the DVE norm-evict chain) into the [dh,
  token] layout phase C needs.
- Phase C partials are staged per 512-token chunk in SBUF (bf16) and
  stored with two DMAs per chunk.
- Emission follows an explicit round schedule that pairs ACT-heavy
  attention chunks (large causal prefixes) with PE-heavy projection
  chunks, processing batch 1's chunks in the order c1, c3, c2, c0 so the
  tail rounds are light.
"""

import numpy as np
from contextlib import ExitStack

import ml_dtypes
import concourse.bass as bass
import concourse.bacc as bacc
import concourse.mybir as mybir
import concourse.tile as tile
from concourse import bass_utils

F32 = mybir.dt.float32
BF16 = mybir.dt.bfloat16
AF = mybir.ActivationFunctionType

B, T, E = 2, 2048, 1024
NH, DH = 16, 64
NCORES = 8
HPC = NH // NCORES          # heads per core = 2
BT = B * T                  # 4096 tokens total
TCH = 512                   # token chunk
NTC = BT // TCH             # 8 token chunks
NE = E // 128               # 8 contraction tiles over E
ST = T // 128               # 16 key tiles per batch
CPB = T // TCH              # 4 query chunks per batch
SCALE = 1.0 / 8.0           # 1/sqrt(DH)


def _emit(tc, yT, xT, wqk, wv, bqk, bv, wp, bp, trid, identd):
    nc = tc.nc
    ctx = tc._ctx

    singles = ctx.enter_context(tc.tile_pool(name="singles", bufs=1))
    xpool = ctx.enter_context(tc.tile_pool(name="xpool", bufs=3))
    epool = ctx.enter_context(tc.tile_pool(name="epool", bufs=8))
    apool = ctx.enter_context(tc.tile_pool(name="apool", bufs=12))
    rpool = ctx.enter_context(tc.tile_pool(name="rpool", bufs=8))
    ypool = ctx.enter_context(tc.tile_pool(name="ypool", bufs=2))
    psum = ctx.enter_context(tc.tile_pool(name="psum", space="PSUM", bufs=2))

    # --- persistent tiles; only wqk is loaded up front (x0 races it) ---
    wqk_sb = singles.tile([128, NE, 2 * 128], BF16)
    nc.scalar.dma_start(out=wqk_sb,
                        in_=wqk.rearrange("(e p) m -> p e m", p=128))
    wv_sb = singles.tile([128, NE, 128], BF16)
    bqk_sb = singles.tile([128, 2], F32)
    bv_sb = singles.tile([1, 128], BF16)
    bp_sb = singles.tile([128, NE], F32)
    tri_sb = singles.tile([128, 128], BF16)
    id_sb = singles.tile([128, 128], BF16)
    wp_sb = singles.tile([128, E], BF16)
    ones_sb = singles.tile([1, 128], BF16)
    nc.gpsimd.memset(ones_sb, 1.0)

    def load_consts():
        nc.scalar.dma_start(out=wv_sb,
                            in_=wv.rearrange("(e p) m -> p e m", p=128))
        nc.scalar.dma_start(out=bqk_sb,
                            in_=bqk.rearrange("(c p) -> p c", p=128))
        nc.scalar.dma_start(out=bv_sb, in_=bv)
        nc.scalar.dma_start(out=tri_sb, in_=trid)
        nc.scalar.dma_start(out=id_sb, in_=identd)

    def load_wp():
        nc.scalar.dma_start(out=wp_sb, in_=wp)
        nc.scalar.dma_start(out=bp_sb, in_=bp.rearrange("(c p) -> p c", p=128))

    qT = singles.tile([128, BT], BF16)   # rows: 2 heads x 64 dh
    kT = singles.tile([128, BT], BF16)
    aoT = singles.tile([128, BT], BF16)  # normalized attn-out^T
    # V natural layout per (batch, s-tile, head): [s-tok, dh] + ones col 64
    v1 = singles.tile([128, B, ST, HPC, 65], BF16)
    nc.gpsimd.memset(v1[:, :, :, :, 64:65], 1.0)

    xchunks = [None] * NTC

    # ---------------- phase A units (projection of one 512-token chunk) ----
    def make_a_units(g):
        units = []
        halves = 2 if g == 0 else 1

        def load_x(lo, hi):
            def unit():
                if xchunks[g] is None:
                    xchunks[g] = xpool.tile([128, NE, TCH], BF16, tag="xch",
                                            name=f"xch{g}")
                nc.sync.dma_start(
                    out=xchunks[g][:, :, lo:hi],
                    in_=xT.rearrange("(e p) t -> p e t", p=128)[
                        :, :, g * TCH + lo:g * TCH + hi],
                )
            return unit

        def qk(m, lo, hi):
            def unit():
                xch = xchunks[g]
                ps = psum.tile([128, hi - lo], F32, tag="mm", bufs=4,
                               name=f"psA{g}_{m}_{lo}")
                for e in range(NE):
                    nc.tensor.matmul(
                        ps,
                        lhsT=wqk_sb[:, e, m * 128:(m + 1) * 128],
                        rhs=xch[:, e, lo:hi],
                        start=(e == 0),
                        stop=(e == NE - 1),
                    )
                dst = qT if m == 0 else kT
                sl = dst[:, g * TCH + lo:g * TCH + hi]
                if m == 0:
                    nc.scalar.activation(sl, ps, AF.Identity,
                                         bias=bqk_sb[:, 0:1])
                else:
                    nc.vector.tensor_scalar_add(sl, ps, bqk_sb[:, 1:2])
            return unit

        bidx, cc = divmod(g, CPB)

        def vtile(stl):
            def unit():
                xch = xchunks[g]
                s_idx = cc * 4 + stl
                ps = psum.tile([128, 2, 64], F32, tag="aux", bufs=2,
                               name=f"psV{g}_{stl}")
                for e in range(NE):
                    nc.tensor.matmul(
                        ps,
                        lhsT=xch[:, e, stl * 128:(stl + 1) * 128],
                        rhs=wv_sb[:, e, :],
                        start=(e == 0), stop=False,
                    )
                # += ones^T (1x128) @ bv (1x128): broadcast bias over tokens
                nc.tensor.matmul(ps, lhsT=ones_sb, rhs=bv_sb,
                                 start=False, stop=True)
                nc.vector.tensor_copy(v1[:, bidx, s_idx, :, 0:64], ps)
            return unit

        if halves == 2:
            units.append(load_x(0, 256))
            units.append(load_consts)
            units.append(qk(0, 0, 256))
            units.append(load_x(256, 512))
            units.append(qk(1, 0, 256))
            units.append(vtile(0))
            units.append(vtile(1))
            units.append(qk(0, 256, 512))
            units.append(qk(1, 256, 512))
            units.append(vtile(2))
            units.append(vtile(3))
        else:
            units.append(load_x(0, 512))
            units.append(qk(0, 0, 512))
            units.append(qk(1, 0, 512))
            for stl in range(4):
                units.append(vtile(stl))
        return units

    # ---------------- phase B units (attention for one chunk) --------------
    def make_b_units(g):
        bidx, cc = divmod(g, CPB)
        units = []
        smax = 4 * cc + 3
        ao_nat = [None] * 4  # per q-subtile [128, HPC, 64] bf16

        def alloc_ao(j):
            def unit():
                ao_nat[j] = apool.tile([128, HPC, 64], BF16, tag="ao",
                                       name=f"ao{g}_{j}")
            return unit

        def head_work(hh):
            hs = slice(hh * 64, (hh + 1) * 64)
            avq = [None]
            pend = []  # (et_tile, col_off, goff, s) awaiting AV

            def alloc():
                avq[0] = psum.tile([128, 4, 65], F32, tag="avq", bufs=2,
                                   name=f"avq{g}_{hh}")

            def score_off(s):
                # off-diagonal key block, full query width
                def unit():
                    sp = psum.tile([128, TCH], F32, tag="mm", bufs=4,
                                   name=f"psS{g}_{hh}_{s}")
                    nc.tensor.matmul(
                        sp,
                        lhsT=kT[hs, bidx * T + s * 128:
                                bidx * T + (s + 1) * 128],
                        rhs=qT[hs, bidx * T + cc * TCH:
                               bidx * T + (cc + 1) * TCH],
                        start=True, stop=True,
                    )
                    et = epool.tile([128, TCH], BF16, tag="et",
                                    name=f"et{g}_{hh}_{s}")
                    nc.scalar.activation(et, sp, AF.Exp, scale=SCALE)
                    pend.append((et, 0, 0, s))
                return unit

            def score_diag(s):
                def unit():
                    j = s - 4 * cc
                    goff = 128 * j
                    w = TCH - goff
                    sp = psum.tile([128, TCH], F32, tag="mm", bufs=4,
                                   name=f"psS{g}_{hh}_{s}")
                    nc.tensor.matmul(
                        sp[:, 0:w],
                        lhsT=kT[hs, bidx * T + s * 128:
                                bidx * T + (s + 1) * 128],
                        rhs=qT[hs, bidx * T + cc * TCH + goff:
                               bidx * T + (cc + 1) * TCH],
                        start=True, stop=True,
                    )
                    et = epool.tile([128, TCH], BF16, tag="et",
                                    name=f"et{g}_{hh}_{s}")
                    nc.scalar.activation(et[:, 0:w], sp[:, 0:w], AF.Exp,
                                         scale=SCALE)
                    # zero the masked triangle (q < key) of the first 128
                    # computed columns, on the idle GPSIMD engine
                    nc.gpsimd.tensor_mul(et[:, 0:128], et[:, 0:128], tri_sb)
                    pend.append((et, 0, goff, s))
                return unit

            def av(idx):
                def unit():
                    et, coff, goff, s = pend[idx]
                    jlo = goff // 128
                    for j in range(jlo, 4):
                        nc.tensor.matmul(
                            avq[0][:, j, :],
                            lhsT=et[:, coff + j * 128 - goff:
                                    coff + (j + 1) * 128 - goff],
                            rhs=v1[:, bidx, s, hh, :],
                            start=(s == 0 and j == 0),
                            stop=(s == smax and j == 3),
                        )
                return unit

            def norm(j):
                def unit():
                    r = rpool.tile([128, 1], F32, tag="r",
                                   name=f"r{g}_{hh}_{j}")
                    nc.vector.reciprocal(r, avq[0][:, j, 64:65])
                    if g in (6, 4) and j % 2 == 0:
                        nc.scalar.mul(ao_nat[j][:, hh, :],
                                      avq[0][:, j, 0:64], r)
                    else:
                        nc.vector.tensor_scalar_mul(
                            ao_nat[j][:, hh, :], avq[0][:, j, 0:64], r)
                return unit

            return alloc, score_off, score_diag, av, norm

        h_ctx = [head_work(0), head_work(1)]
        units.append(h_ctx[0][0])
        units.append(h_ctx[1][0])
        for j in range(4):
            units.append(alloc_ao(j))
        # off-diagonal pairs then diagonal blocks; per-head streams
        # interleaved with one pending-AV of lag for pipelining
        navail = [0, 0]   # pend entries produced per head
        ndone = [0, 0]    # AV units emitted per head

        def emit_av_upto(hh, upto):
            while ndone[hh] < upto:
                units.append(h_ctx[hh][3](ndone[hh]))
                ndone[hh] += 1

        for s in range(0, 4 * cc):
            for hh in (0, 1):
                units.append(h_ctx[hh][1](s))
                navail[hh] += 1
                emit_av_upto(hh, navail[hh] - 1)
        for s in range(4 * cc, smax + 1):
            for hh in (0, 1):
                units.append(h_ctx[hh][2](s))
                navail[hh] += 1
                emit_av_upto(hh, navail[hh] - 1)
        for hh in (0, 1):
            emit_av_upto(hh, navail[hh])
        for j in range(4):
            for hh in (0, 1):
                units.append(h_ctx[hh][4](j))

        # transpose ao_nat -> aoT (phase C layout); deferred one round so
        # PE doesn't stall on the DVE norm-evict chain at chunk end
        def transp(j):
            def unit():
                tp = psum.tile([128, 128], BF16, tag="aux", bufs=2,
                               name=f"tp{g}_{j}")
                nc.tensor.transpose(tp, ao_nat[j], id_sb)
                nc.vector.tensor_copy(
                    aoT[:, g * TCH + j * 128:g * TCH + (j + 1) * 128], tp)
            return unit
        tails = [transp(j) for j in range(4)]
        return units, tails

    # ---------------- phase C units (output projection of one chunk) -------
    def make_c_units(g, last):
        units = []
        ysb = [None]

        def alloc():
            ysb[0] = ypool.tile([128, NE, TCH], BF16, tag="ysb",
                                name=f"ysb{g}")
        units.append(alloc)

        def oc_unit(oc):
            def unit():
                ps = psum.tile([128, TCH], F32, tag="mm", bufs=4,
                               name=f"psC{g}_{oc}")
                nc.tensor.matmul(
                    ps,
                    lhsT=wp_sb[:, oc * 128:(oc + 1) * 128],
                    rhs=aoT[:, g * TCH:(g + 1) * TCH],
                    start=True, stop=True,
                )
                if last and oc % 2 == 0:
                    nc.scalar.activation(ysb[0][:, oc, :], ps, AF.Identity,
                                         bias=bp_sb[:, oc:oc + 1])
                else:
                    nc.vector.tensor_scalar_add(ysb[0][:, oc, :], ps,
                                                bp_sb[:, oc:oc + 1])
            return unit
        for oc in range(NE):
            units.append(oc_unit(oc))

        def store(olo, ohi):
            def unit():
                nc.sync.dma_start(
                    out=yT.rearrange("(o p) t -> p o t", p=128)[
                        :, olo:ohi, g * TCH:(g + 1) * TCH],
                    in_=ysb[0][:, olo:ohi, :])
            return unit
        if last:
            for o in range(0, NE, 2):
                units.append(store(o, o + 2))
        else:
            units.append(store(0, 4))
            units.append(store(4, NE))
        return units

    # ---------------- schedule -------------------------------------------
    def interleave(*streams, prime=0):
        """Round-robin proportional merge. `prime` emits that many units of
        the LAST stream first (used to start a B-chunk's score->exp pipeline
        immediately at a round boundary so ACT never goes dry)."""
        streams = [list(s) for s in streams if s]
        total = sum(len(s) for s in streams)
        out = []
        idx = [0] * len(streams)
        if prime and streams:
            k = min(prime, len(streams[-1]))
            out.extend(streams[-1][:k])
            idx[-1] = k
        for _ in range(total - len(out)):
            best, bestv = None, -1.0
            for i, s in enumerate(streams):
                rem = len(s) - idx[i]
                if rem <= 0:
                    continue
                frac = rem / len(s)
                if frac > bestv:
                    best, bestv = i, frac
            out.append(streams[best][idx[best]])
            idx[best] += 1
        return out

    # rounds: (A-chunks, B-chunk or None, TC-chunk or None)
    # batch 1's B chunks run in order c1, c3, c2, c0 so ACT-heavy chunks
    # get projection filler and the tail rounds are light.
    SCHED = [
        ([0, 1], 0, None),
        ([2],    1, None),
        ([3],    2, 0),
        ([4, 5], 3, 1),
        ([6, 7], 5, 2),
        ([],     7, 3),
        ([],     6, 5),
    ]
    b_tails = {}
    emitted_wp = False
    for rnd, (a_list, b_chunk, tc_chunk) in enumerate(SCHED):
        streams = []
        b_stream = None
        if b_chunk is not None:
            b_units, tails = make_b_units(b_chunk)
            b_tails[b_chunk] = tails
            b_stream = b_units
        if rnd == 0:
            # B0 reads the full qT/kT chunk 0: keep it strictly after A0
            streams.append(make_a_units(a_list[0]) + (b_stream or []))
            for a in a_list[1:]:
                streams.append(make_a_units(a))
        else:
            for a in a_list:
                streams.append(make_a_units(a))
            if not emitted_wp and b_chunk is not None and b_chunk >= 1:
                streams.append([load_wp])
                emitted_wp = True
            if b_stream is not None:
                streams.append(b_stream)
        if tc_chunk is not None:
            # TC stream before B so B stays last for priming
            streams.insert(-1 if b_stream is not None and rnd > 0 else
                           len(streams),
                           b_tails.pop(tc_chunk) + make_c_units(tc_chunk,
                                                                False))
        for u in interleave(*streams,
                            prime=10 if (b_stream is not None and rnd > 0)
                            else 0):
            u()

    # tail rounds: keep per-chunk rounds (slow-dependency units at the
    # head of an in-order queue block ready work, so don't merge them)
    b4_units, b4_tails = make_b_units(4)
    for u in interleave(b4_units, b_tails.pop(7) + make_c_units(7, True)):
        u()
    for u in interleave(b_tails.pop(6) + make_c_units(6, True)):
        u()
    for u in interleave(b4_tails + make_c_units(4, True)):
        u()


def build_bass():
    nc = bacc.Bacc("TRN2", target_bir_lowering=False, debug=False,
                   enable_asserts=False, num_devices=NCORES)
    xT = nc.dram_tensor("xT", [E, BT], BF16, kind="ExternalInput").ap()
    wqk = nc.dram_tensor("wqk", [E, 2 * 128], BF16, kind="ExternalInput").ap()
    wv = nc.dram_tensor("wv", [E, 128], BF16, kind="ExternalInput").ap()
    bqk = nc.dram_tensor("bqk", [2 * 128], F32, kind="ExternalInput").ap()
    bv = nc.dram_tensor("bv", [1, 128], BF16, kind="ExternalInput").ap()
    wp = nc.dram_tensor("wp", [128, E], BF16, kind="ExternalInput").ap()
    bp = nc.dram_tensor("bp", [E], F32, kind="ExternalInput").ap()
    trid = nc.dram_tensor("trid", [128, 128], BF16, kind="ExternalInput").ap()
    identd = nc.dram_tensor("identd", [128, 128], BF16,
                            kind="ExternalInput").ap()
    yT = nc.dram_tensor("yT", [E, BT], BF16, kind="ExternalOutput").ap()
    with tile.TileContext(nc) as tc:
        with nc.allow_low_precision(reason="bf16 operand production"):
            with ExitStack() as ctx:
                tc._ctx = ctx
                _emit(tc, yT, xT, wqk, wv, bqk, bv, wp, bp, trid, identd)
    nc.compile()
    return nc


def make_in_maps(inputs):
    stacked = np.asarray(inputs["stacked"], dtype=np.float32)
    w_attn = np.asarray(inputs["w_attn"], dtype=np.float32)
    b_attn = np.asarray(inputs["b_attn"], dtype=np.float32)
    w_proj = np.asarray(inputs["w_proj"], dtype=np.float32)
    b_proj = np.asarray(inputs["b_proj"], dtype=np.float32)

    bf = ml_dtypes.bfloat16
    xT = np.ascontiguousarray(stacked.reshape(BT, E).T).astype(bf)
    # keep-mask (1.0 where local query >= local key) for diagonal triangles
    rr = np.arange(128)
    tri = (rr[None, :] >= rr[:, None]).astype(np.float32).astype(bf)
    ident = np.eye(128, dtype=np.float32).astype(bf)

    in_maps = []
    for c in range(NCORES):
        lo = c * HPC * DH
        hi = lo + HPC * DH
        wqk_c = np.concatenate(
            [w_attn[:, lo:hi], w_attn[:, E + lo:E + hi]], axis=1)
        wv_c = w_attn[:, 2 * E + lo:2 * E + hi]
        bqk_c = np.concatenate([b_attn[lo:hi], b_attn[E + lo:E + hi]])
        bv_c = b_attn[2 * E + lo:2 * E + hi].reshape(1, 128)
        in_maps.append({
            "xT": xT,
            "wqk": np.ascontiguousarray(wqk_c).astype(bf),
            "wv": np.ascontiguousarray(wv_c).astype(bf),
            "bqk": np.ascontiguousarray(bqk_c),
            "bv": np.ascontiguousarray(bv_c).astype(bf),
            "wp": np.ascontiguousarray(w_proj[lo:hi, :]).astype(bf),
            "bp": b_proj if c == 0 else np.zeros_like(b_proj),
            "trid": tri,
            "identd": ident,
        })
    return in_maps


_NC = None


def _get_nc():
    global _NC
    if _NC is None:
        _NC = build_bass()
    return _NC


def run(inputs, trace=False):
    nc = _get_nc()
    in_maps = make_in_maps(inputs)
    res = bass_utils.run_bass_kernel_spmd(
        nc, in_maps, core_ids=list(range(NCORES)), trace=trace)
    acc = np.zeros((E, BT), dtype=np.float32)
    for out_map in res.results:
        acc += np.asarray(out_map["yT"]).astype(np.float32)
    y = np.ascontiguousarray(acc.T).reshape(B, T, E).astype(np.float32)
    return y, res


def kernel(**inputs):
    y, _ = run(inputs)
    return y
